# revision 13
# baseline (speedup 1.0000x reference)
# BitStackLinear Trainium2 kernel (8-core column-parallel).
#
# reference computation:
#   sign  = unpack_bits(qweight) in {-1,+1}            [4, 4096, 4096]  (b, o, i)
#   w     = sum_b sign_b * (u_b @ vt_b)                [4096, 4096]     (o, i)
#   out   = x @ w.T                                    [4, 2048, 4096]
#
# Sharding: column-parallel over out_features (512 per core). x replicated.
#
# Per-core device program, split-K two-pass so that w.T formation (DVE-bound,
# ~100us) gates as little PE time as possible:
#   Formation (i-tile pairs): L_b.T = vt_b.T @ u_b.T on PE (K=16) -> fp16;
#     sign masks {0,0x8000} via DVE tensor_scalar(AND, SHL) from host-packed
#     INVERTED bits; prod_b = L_b XOR m_b (flips fp16 sign bit -> exact +-L);
#     wT_tile = sum of 4 planes (DVE adds).
#   Pass 1: out1[t, o] = sum_{i<2048} xT.T @ wT  for ALL tokens, psum -> fp16
#     -> DRAM scratch. Only the first token group waits on formation.
#   Pass 2: psum := I.T @ out1 (identity matmul re-injects the partial), then
#     accumulate i>=2048, -> final out.
#   PSUM: formation window runs 6-bank token groups + 2 cycling L banks.
#
# Host prep: transpose x to [in_f, tokens]; repack qweight bits into uint16
# words so that on-device bit l of word j covers output column o = 32*l + j
# (bit-plane-major -> unpacked masks land contiguous in o, no permutation).

import sys

import numpy as np

for p in ("/opt/trn_rl_repo", "/opt/pypackages"):
    if p not in sys.path:
        sys.path.insert(0, p)

import concourse.bacc as bacc
import concourse.mybir as mybir
import concourse.tile as tile
from concourse.bass_utils import run_bass_kernel_spmd

W_BIT, OUT_F, IN_F, K = 4, 4096, 4096, 16
B, S = 4, 2048
T = B * S                      # 8192 tokens
NCORES = 8
OS = OUT_F // NCORES           # 512 out features per core
N_ITILES = IN_F // 128         # 32
HK = N_ITILES // 2             # 16 i-tiles per K-half

# pass-1 token groups: (start_token, n_ttiles). The first four run inside the
# formation window with 6 psum banks (2 reserved for L); the rest use 8.
GROUPS_P1 = [(768 * g, 6) for g in range(4)] + [
    (3072 + 1024 * g, 8) for g in range(5)
]
GROUPS_P2 = [(1024 * g, 8) for g in range(8)]

FP16 = mybir.dt.float16
F32 = mybir.dt.float32
U16 = mybir.dt.uint16
Alu = mybir.AluOpType

_cached = {}


def build_nc():
    nc = bacc.Bacc("TRN2", target_bir_lowering=False, debug=False,
                   num_devices=NCORES)
    xt_p = nc.dram_tensor("xt", [IN_F, T], FP16, kind="ExternalInput").ap()
    qp_p = nc.dram_tensor("qp", [IN_F, 128], U16, kind="ExternalInput").ap()
    ut_p = nc.dram_tensor("ut", [W_BIT, K, OS], FP16, kind="ExternalInput").ap()
    vt_p = nc.dram_tensor("vt4", [W_BIT, K, IN_F], FP16, kind="ExternalInput").ap()
    eye_p = nc.dram_tensor("eye", [128, 128], FP16, kind="ExternalInput").ap()
    out_p = nc.dram_tensor("out", [T, OS], FP16, kind="ExternalOutput").ap()
    oscr = nc.dram_tensor("oscr", [T, OS], FP16).ap()  # pass-1 partials

    with tile.TileContext(nc) as tc:
        with (
            tc.tile_pool(name="const", bufs=1) as cpool,
            tc.tile_pool(name="wt", bufs=1) as wtpool,
            tc.tile_pool(name="fq", bufs=8) as fq,
            tc.tile_pool(name="fl", bufs=3) as fl,
            tc.tile_pool(name="fbits", bufs=3) as fb,
            tc.tile_pool(name="fa", bufs=3) as fa,
            tc.tile_pool(name="mx", bufs=6) as mx,
            tc.tile_pool(name="mo", bufs=8) as mo,
            tc.tile_pool(name="mo1", bufs=8) as mo1,
        ):
            # resident operands
            vt_b = []
            ut_b = []
            for b in range(W_BIT):
                v = cpool.tile([K, IN_F], FP16, tag=f"vt{b}")
                nc.sync.dma_start(v[:], vt_p[b, :, :])
                vt_b.append(v)
                uu = cpool.tile([K, OS], FP16, tag=f"ut{b}")
                nc.sync.dma_start(uu[:], ut_p[b, :, :])
                ut_b.append(uu)
            eye = cpool.tile([128, 128], FP16, tag="eye")
            nc.sync.dma_start(eye[:], eye_p[:])

            wts = [
                wtpool.tile([128, OS], FP16, tag=f"wt{it}", name=f"wt_{it}")
                for it in range(N_ITILES)
            ]

            def emit_formation_pair(ip, psL):
                """Form wT tiles 2*ip and 2*ip+1."""
                it0 = 2 * ip
                q = fq.tile([128, 256], U16, tag="q", name=f"q_{ip}")
                for h in range(2):
                    r0 = (it0 + h) * 128
                    nc.sync.dma_start(
                        q[:, h * 128:(h + 1) * 128], qp_p[r0:r0 + 128, :]
                    )

                # low-rank psums -> fp16 (8 single-matmul rounds, 2 L banks)
                ls = fl.tile([128, 2 * W_BIT * OS], FP16, tag="Ls")
                for r in range(8):
                    h, b = divmod(r, W_BIT)
                    isl = slice((it0 + h) * 128, (it0 + h) * 128 + 128)
                    pl = psL.tile([128, OS], F32, tag="pl", name=f"pl_{ip}_{r}")
                    nc.tensor.matmul(
                        pl[:], vt_b[b][:, isl], ut_b[b][:], start=True, stop=True
                    )
                    nc.scalar.copy(ls[:, r * OS:(r + 1) * OS], pl[:])

                # sign masks in {0, 0x8000}: one tensor_scalar per bit l,
                # covering both halves and all 4 planes (FD=256)
                masks = fb.tile([128, 2 * W_BIT * OS], U16, tag="masks")
                q3 = q[:].rearrange("p (h b j) -> p h b j", h=2, b=W_BIT)
                m5 = masks[:].rearrange(
                    "p (h b l j) -> p h b l j", h=2, b=W_BIT, l=16
                )
                for l in range(16):
                    nc.vector.tensor_scalar(
                        m5[:, :, :, l, :], q3, 1 << l, 15 - l,
                        op0=Alu.bitwise_and, op1=Alu.logical_shift_left,
                    )

                # prod = L ^ m for both halves, all planes: one wide op
                prods = fa.tile([128, 2 * W_BIT * OS], FP16, tag="prods")
                nc.vector.tensor_tensor(
                    prods[:].bitcast(U16), ls[:].bitcast(U16), masks[:],
                    op=Alu.bitwise_xor,
                )
                # wT = p0 + p1 + p2 + p3 per half
                for h in range(2):
                    base = h * W_BIT * OS
                    p01 = fa.tile([128, 2 * OS], FP16, tag="p01")
                    nc.vector.tensor_add(
                        p01[:], prods[:, base:base + 2 * OS],
                        prods[:, base + 2 * OS:base + 4 * OS],
                    )
                    nc.vector.tensor_add(
                        wts[it0 + h][:], p01[:, 0:OS], p01[:, OS:2 * OS]
                    )

            def mm_group(accs, t0, ntt, it, start, stop):
                xs = mx.tile([128, ntt * 128], FP16, tag="x")
                nc.sync.dma_start(
                    xs[:], xt_p[it * 128:(it + 1) * 128, t0:t0 + ntt * 128]
                )
                for tt in range(ntt):
                    nc.tensor.matmul(
                        accs[tt][:], xs[:, tt * 128:(tt + 1) * 128], wts[it][:],
                        start=start, stop=stop,
                    )

            def flush_group(accs, t0, ntt, dst):
                for tt in range(ntt):
                    ot = mo.tile([128, OS], FP16, tag="o")
                    nc.scalar.copy(ot[:], accs[tt][:])
                    r0 = t0 + tt * 128
                    nc.sync.dma_start(dst[r0:r0 + 128, :], ot[:])

            # ---- pass 1 (i-tiles 0..15) + formation ----
            with tc.tile_pool(name="mps6", bufs=6, space="PSUM") as mps6:
                with tc.tile_pool(name="psL", bufs=2, space="PSUM") as psL:
                    # group 0: formation-paced
                    t0, ntt = GROUPS_P1[0]
                    accs = [
                        mps6.tile([128, OS], F32, tag="ps", name=f"a1_0_{tt}")
                        for tt in range(ntt)
                    ]
                    for ip in range(HK // 2):
                        emit_formation_pair(ip, psL)
                        for h in range(2):
                            it = 2 * ip + h
                            mm_group(accs, t0, ntt, it, it == 0, it == HK - 1)
                    flush_group(accs, t0, ntt, oscr)
                    # rest of the formation (i-tiles 16..31) emits now so its
                    # L matmuls run while psL is still open
                    for ip in range(HK // 2, N_ITILES // 2):
                        emit_formation_pair(ip, psL)
                    # groups 1..3 (6 banks; formation may still be live)
                    for gi in range(1, 4):
                        t0, ntt = GROUPS_P1[gi]
                        accs = [
                            mps6.tile([128, OS], F32, tag="ps",
                                      name=f"a1_{gi}_{tt}")
                            for tt in range(ntt)
                        ]
                        for it in range(HK):
                            mm_group(accs, t0, ntt, it, it == 0, it == HK - 1)
                        flush_group(accs, t0, ntt, oscr)

            with tc.tile_pool(name="mps8", bufs=8, space="PSUM") as mps8:
                for gi in range(4, len(GROUPS_P1)):
                    t0, ntt = GROUPS_P1[gi]
                    accs = [
                        mps8.tile([128, OS], F32, tag="ps", name=f"a1_{gi}_{tt}")
                        for tt in range(ntt)
                    ]
                    for it in range(HK):
                        mm_group(accs, t0, ntt, it, it == 0, it == HK - 1)
                    flush_group(accs, t0, ntt, oscr)

                # ---- pass 2 (i-tiles 16..31), merging pass-1 partials ----
                for gi, (t0, ntt) in enumerate(GROUPS_P2):
                    accs = [
                        mps8.tile([128, OS], F32, tag="ps", name=f"a2_{gi}_{tt}")
                        for tt in range(ntt)
                    ]
                    # inject partials: psum := I.T @ out1
                    for tt in range(ntt):
                        o1 = mo1.tile([128, OS], FP16, tag="o1")
                        r0 = t0 + tt * 128
                        nc.sync.dma_start(o1[:], oscr[r0:r0 + 128, :])
                        nc.tensor.matmul(
                            accs[tt][:], eye[:], o1[:], start=True, stop=False
                        )
                    for it in range(HK, N_ITILES):
                        mm_group(accs, t0, ntt, it, False, it == N_ITILES - 1)
                    flush_group(accs, t0, ntt, out_p)
    nc.compile()
    return nc


def prep_inputs(x, qweight, u, vt):
    """Host-side shard prep. Returns per-core input maps."""
    x = np.asarray(x, dtype=np.float16)
    qweight = np.asarray(qweight)
    u = np.asarray(u, dtype=np.float16)
    vt = np.ascontiguousarray(np.asarray(vt, dtype=np.float16))

    xt = np.ascontiguousarray(x.reshape(T, IN_F).T)  # [IN_F, T]

    # unpack bits: (b, o, i); INVERT so mask=0x8000 <=> sign -1 (bit 0)
    bytes_ = qweight.astype(np.uint8)
    bits = np.unpackbits(bytes_.reshape(W_BIT, -1, 1), axis=2, bitorder="little")
    bits = bits.reshape(W_BIT, OUT_F, IN_F)
    inv = (1 - bits).astype(np.uint16)
    # word[c][i, b*32 + j] bit l = inv[b, 512c + 32l + j, i]
    bl = inv.reshape(W_BIT, NCORES, 16, 32, IN_F)  # [b, c, l, j, i]
    words = np.zeros((W_BIT, NCORES, 32, IN_F), np.uint16)
    for l in range(16):
        words |= bl[:, :, l, :, :] << np.uint16(l)
    qp_all = words.transpose(1, 3, 0, 2)  # [c, i, b, j]

    eye = np.eye(128, dtype=np.float16)
    in_maps = []
    for c in range(NCORES):
        uc = u[:, c * OS:(c + 1) * OS, :]                 # [4, 512, 16]
        ut = np.ascontiguousarray(uc.transpose(0, 2, 1))  # [4, 16, 512]
        qp_c = np.ascontiguousarray(qp_all[c]).reshape(IN_F, 128)
        in_maps.append(
            {"xt": xt, "qp": qp_c, "ut": ut, "vt4": vt, "eye": eye}
        )
    return in_maps


def kernel(x, qweight, u, vt, _trace=False):
    if "nc" not in _cached:
        _cached["nc"] = build_nc()
    nc = _cached["nc"]
    in_maps = prep_inputs(x, qweight, u, vt)
    res = run_bass_kernel_spmd(nc, in_maps, list(range(NCORES)), trace=_trace)
    _cached["last_result"] = res
    out = np.concatenate([res.results[c]["out"] for c in range(NCORES)], axis=1)
    return out.reshape(B, S, OUT_F).astype(np.float16)
